# revision 10
# baseline (speedup 1.0000x reference)
"""Trainium2 Bass kernel for nn_DotAttentionUnit.

Reference computation (per batch b):
    h_mul[p,q,h] = hq[q,h] * hp[p,h]
    s_w = tanh(h_mul @ W.T)            # [p,q,v]
    s[p,q] = s_w . v_w                 # reduce over v
    a = softmax(s, axis=q)
    out[p,h] = sum_q a[p,q] * hq[q,h]

Shapes: B=4, LQ=256, LP=256, H=512, V=512.

Sharding: pure data parallel over (b, p-block): 8 cores = 4 batches x 2
p-blocks of 128. Each core computes out[b, pblk:pblk+128, :]. No
collectives.

Per-core device algorithm (PE-bound):
  for p in 0..127:
    scaled[k] = hqT[k] * hpT[k][:, p]          (GpSimd, per-partition scalar)
    psum[m]   = sum_k scaled[k][:,m*128:].T @ WT[k]   (PE, f32r full rate)
    tw[m]     = tanh(psum[m])                  (ScalarE, PSUM->SBUF)
    scores[m][:, p] = sum_v tw[m]*vw           (VectorE fused mul+reduce)
  epilogue: transpose scores -> softmax over q -> exp^T @ hq -> scale by
  1/sum -> DMA out.
"""

import numpy as np

B, LQ, LP, H, V = 4, 256, 256, 512, 512
NCORES = 8
PB = 128  # p rows per core
KH = H // 128  # 4 contraction tiles
MQ = LQ // 128  # 2 q tiles

_CACHED_NC = None


def _build_nc():
    from contextlib import ExitStack

    import concourse.bass as bass
    import concourse.mybir as mybir
    import concourse.tile as tile
    from concourse import bacc
    from concourse.masks import make_identity

    f32 = mybir.dt.float32
    f32r = mybir.dt.float32r
    f16 = mybir.dt.float16
    AF = mybir.ActivationFunctionType
    ALU = mybir.AluOpType  # noqa: F841

    nc = bacc.Bacc("TRN2", target_bir_lowering=False, debug=False)

    # f32r (TF32) declarations for matmul operands: the BIR verifier
    # requires every producer of an f32r-matmul operand to emit f32r.
    hqT_d = nc.dram_tensor("hqT", [H, LQ], f32, kind="ExternalInput")
    hpT_d = nc.dram_tensor("hpT", [H, PB], f32, kind="ExternalInput")
    WT_d = nc.dram_tensor("WT", [H, V], f32r, kind="ExternalInput")
    vwb_d = nc.dram_tensor("vwb", [128, V], f16, kind="ExternalInput")
    hq_d = nc.dram_tensor("hq", [LQ, H], f32r, kind="ExternalInput")
    out_d = nc.dram_tensor("out", [PB, H], f32, kind="ExternalOutput")

    with tile.TileContext(nc) as tc, ExitStack() as ctx:
        consts = ctx.enter_context(tc.tile_pool(name="consts", bufs=1))
        scaled_pool = ctx.enter_context(tc.tile_pool(name="scaled", bufs=3))
        tanh_pool = ctx.enter_context(tc.tile_pool(name="tanh", bufs=4))
        scratch_pool = ctx.enter_context(tc.tile_pool(name="scratch", bufs=2))
        epi = ctx.enter_context(tc.tile_pool(name="epi", bufs=1))
        psum_main = ctx.enter_context(tc.tile_pool(name="psmain", bufs=4, space="PSUM"))
        psum_tp = ctx.enter_context(tc.tile_pool(name="pstp", bufs=2, space="PSUM"))
        psum_out = ctx.enter_context(tc.tile_pool(name="psout", bufs=1, space="PSUM"))

        hqT_s = consts.tile([128, KH, LQ], f32)
        nc.sync.dma_start(hqT_s[:], hqT_d.ap().rearrange("(k p) q -> p k q", p=128))
        hpT_s = consts.tile([128, KH, PB], f32)
        nc.sync.dma_start(hpT_s[:], hpT_d.ap().rearrange("(k p) q -> p k q", p=128))
        WT_s = consts.tile([128, KH, V], f32r)
        nc.sync.dma_start(WT_s[:], WT_d.ap().rearrange("(k p) v -> p k v", p=128))
        vw_s = consts.tile([128, V], f16)
        nc.sync.dma_start(vw_s[:], vwb_d.ap())
        hq_s = consts.tile([128, MQ, H], f32r)
        nc.sync.dma_start(hq_s[:], hq_d.ap().rearrange("(m p) h -> p m h", p=128))
        ident = consts.tile([128, 128], f32)
        make_identity(nc, ident[:])
        # scores[q, m, p]: column p filled per main-loop iteration
        scores = consts.tile([128, MQ, PB], f32)

        for p in range(PB):
            scaled = scaled_pool.tile([128, KH, LQ], f32r, tag="scaled")
            for k in range(KH):
                nc.gpsimd.tensor_scalar_mul(
                    scaled[:, k, :], hqT_s[:, k, :], hpT_s[:, k, p : p + 1]
                )
            for m in range(MQ):
                ps = psum_main.tile([128, V], f32, tag="ps")
                for k in range(KH):
                    nc.tensor.matmul(
                        ps[:],
                        scaled[:, k, bass.ts(m, 128)],
                        WT_s[:, k, :],
                        start=(k == 0),
                        stop=(k == KH - 1),
                    )
                tw = tanh_pool.tile([128, V], f16, tag="tw")
                nc.scalar.activation(tw[:], ps[:], AF.Tanh)
                sc = scratch_pool.tile([128, V], f16, tag="sc")
                nc.vector.tensor_mul(sc[:], tw[:], vw_s[:])
                if m == 0:
                    nc.vector.reduce_sum(
                        scores[:, m, p : p + 1],
                        sc[:],
                        axis=mybir.AxisListType.X,
                    )
                else:
                    trash = scratch_pool.tile([128, V], f16, tag="trash")
                    nc.scalar.activation(
                        trash[:],
                        sc[:],
                        AF.Identity,
                        accum_out=scores[:, m, p : p + 1],
                    )

        # ---- epilogue: softmax over q + attention-weighted sum of hq ----
        s_pq = epi.tile([128, LQ], f32)  # [p, q]
        for m in range(MQ):
            pst = psum_tp.tile([128, 128], f32, tag="tp")
            nc.tensor.transpose(pst[:], scores[:, m, :], ident[:])
            nc.vector.tensor_copy(s_pq[:, bass.ts(m, 128)], pst[:])
        negmax = epi.tile([128, 1], f32)
        nc.vector.reduce_max(
            negmax[:], s_pq[:], axis=mybir.AxisListType.X, negate=True
        )
        e_t = epi.tile([128, LQ], f32)
        ssum = epi.tile([128, 1], f32)
        nc.scalar.activation(
            e_t[:], s_pq[:], AF.Exp, bias=negmax[:], accum_out=ssum[:]
        )
        rcp = epi.tile([128, 1], f32)
        nc.vector.reciprocal(rcp[:], ssum[:])
        eT = epi.tile([128, MQ, 128], f32r)
        for m in range(MQ):
            pet = psum_tp.tile([128, 128], f32, tag="tp")
            nc.tensor.transpose(pet[:], e_t[:, bass.ts(m, 128)], ident[:])
            nc.vector.tensor_copy(eT[:, m, :], pet[:])
        out_ps = psum_out.tile([128, H], f32, tag="outps")
        for m in range(MQ):
            nc.tensor.matmul(
                out_ps[:],
                eT[:, m, :],
                hq_s[:, m, :],
                start=(m == 0),
                stop=(m == MQ - 1),
            )
        out_s = epi.tile([128, H], f32)
        nc.scalar.activation(out_s[:], out_ps[:], AF.Copy, scale=rcp[:])
        nc.sync.dma_start(out_d.ap(), out_s[:])

    nc.compile()
    return nc


def get_nc():
    global _CACHED_NC
    if _CACHED_NC is None:
        _CACHED_NC = _build_nc()
    return _CACHED_NC


def make_in_maps(hq, hp, W, v_w):
    hq = np.asarray(hq, dtype=np.float32)
    hp = np.asarray(hp, dtype=np.float32)
    W = np.asarray(W, dtype=np.float32)
    v_w = np.asarray(v_w, dtype=np.float32)
    WT = np.ascontiguousarray(W.T)
    vwb = np.ascontiguousarray(
        np.broadcast_to(v_w.reshape(1, V), (128, V))
    ).astype(np.float16)
    in_maps = []
    for c in range(NCORES):
        b = c // 2
        pb = (c % 2) * PB
        in_maps.append(
            {
                "hqT": np.ascontiguousarray(hq[b].T),
                "hpT": np.ascontiguousarray(hp[b, pb : pb + PB].T),
                "WT": WT,
                "vwb": vwb,
                "hq": np.ascontiguousarray(hq[b]),
            }
        )
    return in_maps


def gather_out(results):
    out = np.empty((B, LP, H), np.float32)
    for c in range(NCORES):
        b = c // 2
        pb = (c % 2) * PB
        out[b, pb : pb + PB] = results[c]["out"]
    return out


def kernel(hq, hp, W, v_w):
    from concourse.bass_utils import run_bass_kernel_spmd

    nc = get_nc()
    in_maps = make_in_maps(hq, hp, W, v_w)
    res = run_bass_kernel_spmd(nc, in_maps, core_ids=list(range(NCORES)))
    return gather_out(res.results)


# revision 15
# speedup vs baseline: 1.2293x; 1.2293x over previous
"""Trainium2 Bass kernel for nn_DotAttentionUnit.

Reference computation (per batch b):
    h_mul[p,q,h] = hq[q,h] * hp[p,h]
    s_w = tanh(h_mul @ W.T)            # [p,q,v]
    s[p,q] = s_w . v_w                 # reduce over v
    a = softmax(s, axis=q)
    out[p,h] = sum_q a[p,q] * hq[q,h]

Shapes: B=4, LQ=256, LP=256, H=512, V=512.

Sharding: pure data parallel over (b, p-block): 8 cores = 4 batches x 2
p-blocks of 128. Each core computes out[b, pblk:pblk+128, :]. No
collectives.

Per-core device algorithm (PE-bound, fp16 matmul operands with fp32 PSUM
accumulation; fp16 mantissa ~ TF32, keeps rel err ~1e-4):
  for p in 0..127:
    scaled[k]  = hqT[k] * hpT[k][:, p]     (3 on Pool, 1 on DVE)
    psum[m]    = sum_k scaled[k][:,m*128:].T @ WT[k]  (PE, 8 matmuls N=512)
    tw         = tanh(psum)                (ACT, one [128,1024] op)
    sc         = tw * vw                   (DVE, one wide fp16 mul)
    scores[0][:, p] = reduce(sc[m=0])      (DVE reduce)
    scores[1][:, p] = reduce(sc[m=1])      (ACT Identity + accum_out)
  epilogue: PE-transpose scores -> softmax over q (max/exp/sum fused) ->
  transpose exp -> exp^T @ hq -> scale rows by 1/sum -> DMA out.
"""

import numpy as np

B, LQ, LP, H, V = 4, 256, 256, 512, 512
NCORES = 8
PB = 128  # p rows per core
KH = H // 128  # 4 contraction tiles
MQ = LQ // 128  # 2 q tiles

_CACHED_NC = None


def _build_nc():
    from contextlib import ExitStack

    import concourse.bass as bass
    import concourse.mybir as mybir
    import concourse.tile as tile
    from concourse import bacc
    from concourse.masks import make_identity

    f32 = mybir.dt.float32
    f16 = mybir.dt.float16
    AF = mybir.ActivationFunctionType

    nc = bacc.Bacc("TRN2", target_bir_lowering=False, debug=False)

    hqT_d = nc.dram_tensor("hqT", [H, LQ], f16, kind="ExternalInput")
    hpT_d = nc.dram_tensor("hpT", [H, PB], f32, kind="ExternalInput")
    WT_d = nc.dram_tensor("WT", [H, V], f16, kind="ExternalInput")
    vwb_d = nc.dram_tensor("vwb", [128, MQ * V], f16, kind="ExternalInput")
    hq_d = nc.dram_tensor("hq", [LQ, H], f16, kind="ExternalInput")
    out_d = nc.dram_tensor("out", [PB, H], f32, kind="ExternalOutput")

    with tile.TileContext(nc) as tc, ExitStack() as ctx:
        consts = ctx.enter_context(tc.tile_pool(name="consts", bufs=1))
        scaled_pool = ctx.enter_context(tc.tile_pool(name="scaled", bufs=3))
        tanh_pool = ctx.enter_context(tc.tile_pool(name="tanh", bufs=3))
        scratch_pool = ctx.enter_context(tc.tile_pool(name="scratch", bufs=2))
        epi = ctx.enter_context(tc.tile_pool(name="epi", bufs=1))

        hqT_s = consts.tile([128, KH, LQ], f16)
        nc.sync.dma_start(hqT_s[:], hqT_d.ap().rearrange("(k p) q -> p k q", p=128))
        hpT_s = consts.tile([128, KH, PB], f32)
        nc.sync.dma_start(hpT_s[:], hpT_d.ap().rearrange("(k p) q -> p k q", p=128))
        WT_s = consts.tile([128, KH, V], f16)
        nc.sync.dma_start(WT_s[:], WT_d.ap().rearrange("(k p) v -> p k v", p=128))
        vw_s = consts.tile([128, MQ * V], f16)
        nc.sync.dma_start(vw_s[:], vwb_d.ap())
        hq_s = consts.tile([128, MQ, H], f16)
        nc.sync.dma_start(hq_s[:], hq_d.ap().rearrange("(m p) h -> p m h", p=128))
        ident = consts.tile([128, 128], f32)
        make_identity(nc, ident[:])
        # scores[q, m, p]: column p filled per main-loop iteration
        scores = consts.tile([128, MQ, PB], f32)

        with tc.tile_pool(name="psmain", bufs=3, space="PSUM") as psum_main:
            for p in range(PB):
                scaled = scaled_pool.tile([128, KH, LQ], f16, tag="scaled")
                for k in range(KH):
                    if k == 2:
                        nc.scalar.mul(
                            scaled[:, k, :], hqT_s[:, k, :],
                            hpT_s[:, k, p : p + 1],
                        )
                    else:
                        nc.gpsimd.tensor_scalar_mul(
                            scaled[:, k, :], hqT_s[:, k, :],
                            hpT_s[:, k, p : p + 1],
                        )
                ps = psum_main.tile([128, MQ * V], f32, tag="ps")
                for m in range(MQ):
                    for k in range(KH):
                        nc.tensor.matmul(
                            ps[:, m * V : (m + 1) * V],
                            scaled[:, k, bass.ts(m, 128)],
                            WT_s[:, k, :],
                            start=(k == 0),
                            stop=(k == KH - 1),
                        )
                tw = tanh_pool.tile([128, MQ * V], f16, tag="tw")
                nc.scalar.activation(tw[:], ps[:], AF.Tanh)
                sc = scratch_pool.tile([128, MQ, V], f16, tag="sc")
                nc.vector.tensor_mul(
                    sc[:].rearrange("p m v -> p (m v)"), tw[:], vw_s[:]
                )
                nc.vector.reduce_sum(
                    scores[:, :, p : p + 1], sc[:],
                    axis=mybir.AxisListType.X,
                )

        # ---- epilogue: softmax over q + attention-weighted sum of hq ----
        with (
            tc.tile_pool(name="pstp", bufs=2, space="PSUM") as psum_tp,
            tc.tile_pool(name="psout", bufs=1, space="PSUM") as psum_out,
        ):
            s_pq = epi.tile([128, LQ], f32)  # [p, q]
            for m in range(MQ):
                pst = psum_tp.tile([128, 128], f32, tag="tp")
                nc.tensor.transpose(pst[:], scores[:, m, :], ident[:])
                nc.vector.tensor_copy(s_pq[:, bass.ts(m, 128)], pst[:])
            negmax = epi.tile([128, 1], f32)
            nc.vector.reduce_max(
                negmax[:], s_pq[:], axis=mybir.AxisListType.X, negate=True
            )
            e_t = epi.tile([128, LQ], f32)
            ssum = epi.tile([128, 1], f32)
            nc.scalar.activation(
                e_t[:], s_pq[:], AF.Exp, bias=negmax[:], accum_out=ssum[:]
            )
            rcp = epi.tile([128, 1], f32)
            nc.vector.reciprocal(rcp[:], ssum[:])
            eT = epi.tile([128, MQ, 128], f16)
            for m in range(MQ):
                pet = psum_tp.tile([128, 128], f32, tag="tp")
                nc.tensor.transpose(pet[:], e_t[:, bass.ts(m, 128)], ident[:])
                nc.vector.tensor_copy(eT[:, m, :], pet[:])
            out_ps = psum_out.tile([128, H], f32, tag="outps")
            for m in range(MQ):
                nc.tensor.matmul(
                    out_ps[:],
                    eT[:, m, :],
                    hq_s[:, m, :],
                    start=(m == 0),
                    stop=(m == MQ - 1),
                )
            out_s = epi.tile([128, H], f32)
            nc.scalar.activation(out_s[:], out_ps[:], AF.Copy, scale=rcp[:])
            nc.sync.dma_start(out_d.ap(), out_s[:])

    nc.compile()
    return nc


def get_nc():
    global _CACHED_NC
    if _CACHED_NC is None:
        _CACHED_NC = _build_nc()
    return _CACHED_NC


def make_in_maps(hq, hp, W, v_w):
    hq = np.asarray(hq, dtype=np.float32)
    hp = np.asarray(hp, dtype=np.float32)
    W = np.asarray(W, dtype=np.float32)
    v_w = np.asarray(v_w, dtype=np.float32)
    WT = np.ascontiguousarray(W.T).astype(np.float16)
    vw1 = v_w.reshape(1, V).astype(np.float16)
    vwb = np.ascontiguousarray(
        np.broadcast_to(np.tile(vw1, (1, MQ)), (128, MQ * V))
    )
    in_maps = []
    for c in range(NCORES):
        b = c // 2
        pb = (c % 2) * PB
        in_maps.append(
            {
                "hqT": np.ascontiguousarray(hq[b].T).astype(np.float16),
                "hpT": np.ascontiguousarray(hp[b, pb : pb + PB].T),
                "WT": WT,
                "vwb": vwb,
                "hq": np.ascontiguousarray(hq[b]).astype(np.float16),
            }
        )
    return in_maps


def gather_out(results):
    out = np.empty((B, LP, H), np.float32)
    for c in range(NCORES):
        b = c // 2
        pb = (c % 2) * PB
        out[b, pb : pb + PB] = results[c]["out"]
    return out


def kernel(hq, hp, W, v_w):
    from concourse.bass_utils import run_bass_kernel_spmd

    nc = get_nc()
    in_maps = make_in_maps(hq, hp, W, v_w)
    res = run_bass_kernel_spmd(nc, in_maps, core_ids=list(range(NCORES)))
    return gather_out(res.results)


# revision 26
# speedup vs baseline: 1.2350x; 1.0046x over previous
"""Trainium2 Bass kernel for nn_DotAttentionUnit.

Reference computation (per batch b):
    h_mul[p,q,h] = hq[q,h] * hp[p,h]
    s_w = tanh(h_mul @ W.T)            # [p,q,v]
    s[p,q] = s_w . v_w                 # reduce over v
    a = softmax(s, axis=q)
    out[p,h] = sum_q a[p,q] * hq[q,h]

Shapes: B=4, LQ=256, LP=256, H=512, V=512.

Sharding: pure data parallel over (b, p-block): 8 cores = 4 batches x 2
p-blocks of 128. Each core computes out[b, pblk:pblk+128, :]. No
collectives.

Per-core device algorithm (PE-bound, fp16 matmul operands with fp32 PSUM
accumulation; fp16 mantissa ~ TF32, keeps rel err ~1e-4):
  for p in 0..127:
    scaled[k]  = hqT[k] * hpT[k][:, p]     (2 on Pool, 1 on ACT, 1 on Pool)
    psum[m]    = sum_k scaled[k][:,m*128:].T @ WT[k]  (PE, 8 matmuls N=512)
    tw         = tanh(psum)                (ACT, one [128,1024] op)
    sc         = tw * vw                   (DVE, one wide fp16 mul)
    scores[:, :, p] = reduce(sc)           (DVE, one fused wide reduce)
  epilogue (x2 chunks of 64 p-rows, first chunk overlapped mid-loop):
  PE-transpose scores chunk -> exp+sum (ACT, no max shift needed: |s| is
  small) -> transpose exp -> exp^T @ hq -> scale rows by 1/sum -> DMA out.
"""

import numpy as np

B, LQ, LP, H, V = 4, 256, 256, 512, 512
NCORES = 8
PB = 128  # p rows per core
KH = H // 128  # 4 contraction tiles
MQ = LQ // 128  # 2 q tiles
EPI_CHUNK = 64

_CACHED_NC = None


def _build_nc():
    from contextlib import ExitStack

    import concourse.bass as bass
    import concourse.mybir as mybir
    import concourse.tile as tile
    from concourse import bacc
    from concourse.masks import make_identity

    f32 = mybir.dt.float32
    f16 = mybir.dt.float16
    AF = mybir.ActivationFunctionType

    nc = bacc.Bacc("TRN2", target_bir_lowering=False, debug=False)

    hqT_d = nc.dram_tensor("hqT", [H, LQ], f16, kind="ExternalInput")
    hpT_d = nc.dram_tensor("hpT", [H, PB], f32, kind="ExternalInput")
    WT_d = nc.dram_tensor("WT", [H, V], f16, kind="ExternalInput")
    vwb_d = nc.dram_tensor("vwb", [128, MQ * V], f16, kind="ExternalInput")
    hq_d = nc.dram_tensor("hq", [LQ, H], f16, kind="ExternalInput")
    out_d = nc.dram_tensor("out", [PB, H], f32, kind="ExternalOutput")

    with tile.TileContext(nc) as tc, ExitStack() as ctx:
        consts = ctx.enter_context(tc.tile_pool(name="consts", bufs=1))
        scaled_pool = ctx.enter_context(tc.tile_pool(name="scaled", bufs=3))
        tanh_pool = ctx.enter_context(tc.tile_pool(name="tanh", bufs=3))
        scratch_pool = ctx.enter_context(tc.tile_pool(name="scratch", bufs=2))
        epi = ctx.enter_context(tc.tile_pool(name="epi", bufs=2))
        psum_main = ctx.enter_context(
            tc.tile_pool(name="psmain", bufs=2, space="PSUM")
        )
        psum_tp = ctx.enter_context(tc.tile_pool(name="pstp", bufs=2, space="PSUM"))
        psum_out = ctx.enter_context(tc.tile_pool(name="psout", bufs=1, space="PSUM"))

        # per-k tiles, DMAs interleaved in first-use order; WT rides the
        # gpsimd (SWDGE) queue in parallel with sync's HWDGE loads
        hqT_r = hqT_d.ap().rearrange("(k p) q -> k p q", p=128)
        hpT_r = hpT_d.ap().rearrange("(k p) q -> k p q", p=128)
        WT_r = WT_d.ap().rearrange("(k p) v -> k p v", p=128)
        hpT_s = [consts.tile([128, PB], f32, name=f"hpT{k}") for k in range(KH)]
        hqT_s = [consts.tile([128, LQ], f16, name=f"hqT{k}") for k in range(KH)]
        WT_s = [consts.tile([128, V], f16, name=f"WT{k}") for k in range(KH)]
        # spread the load DMAs over four issue queues so the per-issue
        # overhead doesn't serialize the startup; each queue loads one k-set
        dma_eng = [nc.sync, nc.scalar, nc.sync, nc.scalar]
        for k in range(KH):
            dma_eng[k].dma_start(hpT_s[k][:], hpT_r[k])
            dma_eng[k].dma_start(hqT_s[k][:], hqT_r[k])
            dma_eng[k].dma_start(WT_s[k][:], WT_r[k])
        vw_s = consts.tile([128, MQ * V], f16)
        nc.scalar.dma_start(vw_s[:], vwb_d.ap())
        hq_s = consts.tile([128, MQ, H], f16)
        nc.sync.dma_start(hq_s[:], hq_d.ap().rearrange("(m p) h -> p m h", p=128))
        ident = consts.tile([128, 128], f32)
        make_identity(nc, ident[:])
        # scores[q, m, p]: column p filled per main-loop iteration
        scores = consts.tile([128, MQ, PB], f32)

        def epilogue_chunk(c0, csz):
            """softmax over q + attention output for p-rows [c0, c0+csz)."""
            s_pq = epi.tile([csz, LQ], f32, name=f"s_pq{c0}", tag="s_pq")
            for m in range(MQ):
                pst = psum_tp.tile([csz, 128], f32, tag="tp")
                nc.tensor.transpose(
                    pst[:], scores[:, m, c0 : c0 + csz], ident[:]
                )
                nc.vector.tensor_copy(s_pq[:, bass.ts(m, 128)], pst[:])
            # no max-subtraction: |s| is bounded well inside fp32 exp range
            # for this problem; softmax is shift-invariant so this matches
            # the stable-softmax reference up to rounding
            e_t = epi.tile([csz, LQ], f32, name=f"e_t{c0}", tag="e_t")
            ssum = epi.tile([csz, 1], f32, name=f"ssum{c0}", tag="ssum")
            nc.scalar.activation(e_t[:], s_pq[:], AF.Exp, accum_out=ssum[:])
            rcp = epi.tile([csz, 1], f32, name=f"rcp{c0}", tag="rcp")
            nc.vector.reciprocal(rcp[:], ssum[:])
            eT = epi.tile([128, MQ, csz], f16, name=f"eT{c0}", tag="eT")
            for m in range(MQ):
                pet = psum_tp.tile([128, csz], f32, tag="tp")
                nc.tensor.transpose(
                    pet[:], e_t[:, bass.ts(m, 128)], ident[:csz, :csz]
                )
                nc.vector.tensor_copy(eT[:, m, :], pet[:])
            out_ps = psum_out.tile([csz, H], f32, tag="outps")
            for m in range(MQ):
                nc.tensor.matmul(
                    out_ps[:],
                    eT[:, m, :],
                    hq_s[:, m, :],
                    start=(m == 0),
                    stop=(m == MQ - 1),
                )
            out_s = epi.tile([csz, H], f32, name=f"out_s{c0}", tag="out_s")
            nc.scalar.activation(out_s[:], out_ps[:], AF.Copy, scale=rcp[:])
            nc.sync.dma_start(out_d.ap()[c0 : c0 + csz, :], out_s[:])

        for p in range(PB):
            scaled = [
                scaled_pool.tile([128, LQ], f16, name=f"sc{k}_{p}", tag=f"scl{k}")
                for k in range(KH)
            ]
            for k in range(KH):
                # k=2 prep runs on ACT, except the first few p where ACT is
                # still issuing its share of the input DMAs
                if k == 2 and p >= 3:
                    nc.scalar.mul(
                        scaled[k][:], hqT_s[k][:], hpT_s[k][:, p : p + 1]
                    )
                else:
                    nc.gpsimd.tensor_scalar_mul(
                        scaled[k][:], hqT_s[k][:], hpT_s[k][:, p : p + 1]
                    )
            ps = psum_main.tile([128, MQ * V], f32, tag="ps")
            for m in range(MQ):
                for k in range(KH):
                    nc.tensor.matmul(
                        ps[:, m * V : (m + 1) * V],
                        scaled[k][:, bass.ts(m, 128)],
                        WT_s[k][:],
                        start=(k == 0),
                        stop=(k == KH - 1),
                    )
            tw = tanh_pool.tile([128, MQ * V], f16, tag="tw")
            nc.scalar.activation(tw[:], ps[:], AF.Tanh)
            sc = scratch_pool.tile([128, MQ, V], f16, tag="sc")
            nc.vector.tensor_mul(
                sc[:].rearrange("p m v -> p (m v)"), tw[:], vw_s[:]
            )
            nc.vector.reduce_sum(
                scores[:, :, p : p + 1], sc[:], axis=mybir.AxisListType.X
            )
            if (p + 1) % EPI_CHUNK == 0:
                epilogue_chunk(p + 1 - EPI_CHUNK, EPI_CHUNK)

    nc.compile()
    return nc


def get_nc():
    global _CACHED_NC
    if _CACHED_NC is None:
        _CACHED_NC = _build_nc()
    return _CACHED_NC


def make_in_maps(hq, hp, W, v_w):
    hq = np.asarray(hq, dtype=np.float32)
    hp = np.asarray(hp, dtype=np.float32)
    W = np.asarray(W, dtype=np.float32)
    v_w = np.asarray(v_w, dtype=np.float32)
    WT = np.ascontiguousarray(W.T).astype(np.float16)
    vw1 = v_w.reshape(1, V).astype(np.float16)
    vwb = np.ascontiguousarray(
        np.broadcast_to(np.tile(vw1, (1, MQ)), (128, MQ * V))
    )
    in_maps = []
    for c in range(NCORES):
        b = c // 2
        pb = (c % 2) * PB
        in_maps.append(
            {
                "hqT": np.ascontiguousarray(hq[b].T).astype(np.float16),
                "hpT": np.ascontiguousarray(hp[b, pb : pb + PB].T),
                "WT": WT,
                "vwb": vwb,
                "hq": np.ascontiguousarray(hq[b]).astype(np.float16),
            }
        )
    return in_maps


def gather_out(results):
    out = np.empty((B, LP, H), np.float32)
    for c in range(NCORES):
        b = c // 2
        pb = (c % 2) * PB
        out[b, pb : pb + PB] = results[c]["out"]
    return out


def kernel(hq, hp, W, v_w):
    from concourse.bass_utils import run_bass_kernel_spmd

    nc = get_nc()
    in_maps = make_in_maps(hq, hp, W, v_w)
    res = run_bass_kernel_spmd(nc, in_maps, core_ids=list(range(NCORES)))
    return gather_out(res.results)


# revision 30
# speedup vs baseline: 1.2359x; 1.0007x over previous
"""Trainium2 Bass kernel for nn_DotAttentionUnit.

Reference computation (per batch b):
    h_mul[p,q,h] = hq[q,h] * hp[p,h]
    s_w = tanh(h_mul @ W.T)            # [p,q,v]
    s[p,q] = s_w . v_w                 # reduce over v
    a = softmax(s, axis=q)
    out[p,h] = sum_q a[p,q] * hq[q,h]

Shapes: B=4, LQ=256, LP=256, H=512, V=512.

Sharding: pure data parallel over (b, p-block): 8 cores = 4 batches x 2
p-blocks of 128. Each core computes out[b, pblk:pblk+128, :]. No
collectives.

Per-core device algorithm (PE-bound, fp16 matmul operands with fp32 PSUM
accumulation; fp16 mantissa ~ TF32, keeps rel err ~1e-4):
  for p in 0..127:
    scaled[k]  = hqT[k] * hpT[k][:, p]     (2 on Pool, 1 on ACT, 1 on Pool)
    psum[m]    = sum_k scaled[k][:,m*128:].T @ WT[k]  (PE, 8 matmuls N=512)
    tw         = tanh(psum)                (ACT, one [128,1024] op)
    sc         = tw * vw                   (DVE, one wide fp16 mul)
    scores[:, :, p] = reduce(sc)           (DVE, one fused wide reduce)
  epilogue (x2 chunks of 64 p-rows, first chunk overlapped mid-loop):
  PE-transpose scores chunk -> exp+sum (ACT, no max shift needed: |s| is
  small) -> transpose exp -> exp^T @ hq -> scale rows by 1/sum -> DMA out.
"""

import numpy as np

B, LQ, LP, H, V = 4, 256, 256, 512, 512
NCORES = 8
PB = 128  # p rows per core
KH = H // 128  # 4 contraction tiles
MQ = LQ // 128  # 2 q tiles
EPI_CHUNK = 64

_CACHED_NC = None


def _build_nc(repeat=1):
    from contextlib import ExitStack

    import concourse.bass as bass
    import concourse.mybir as mybir
    import concourse.tile as tile
    from concourse import bacc
    from concourse.masks import make_identity

    f32 = mybir.dt.float32
    f16 = mybir.dt.float16
    AF = mybir.ActivationFunctionType

    nc = bacc.Bacc("TRN2", target_bir_lowering=False, debug=False)

    hqT_d = nc.dram_tensor("hqT", [H, LQ], f16, kind="ExternalInput")
    hpT_d = nc.dram_tensor("hpT", [H, PB], f32, kind="ExternalInput")
    WT_d = nc.dram_tensor("WT", [H, V], f16, kind="ExternalInput")
    vwb_d = nc.dram_tensor("vwb", [128, MQ * V], f16, kind="ExternalInput")
    hq_d = nc.dram_tensor("hq", [LQ, H], f16, kind="ExternalInput")
    out_d = nc.dram_tensor("out", [PB, H], f32, kind="ExternalOutput")

    with tile.TileContext(nc) as tc, ExitStack() as ctx:
        consts = ctx.enter_context(tc.tile_pool(name="consts", bufs=1))
        scaled_pool = ctx.enter_context(tc.tile_pool(name="scaled", bufs=4))
        tanh_pool = ctx.enter_context(tc.tile_pool(name="tanh", bufs=4))
        scratch_pool = ctx.enter_context(tc.tile_pool(name="scratch", bufs=3))
        epi = ctx.enter_context(tc.tile_pool(name="epi", bufs=2))
        psum_main = ctx.enter_context(
            tc.tile_pool(name="psmain", bufs=2, space="PSUM")
        )
        psum_tp = ctx.enter_context(tc.tile_pool(name="pstp", bufs=2, space="PSUM"))
        psum_out = ctx.enter_context(tc.tile_pool(name="psout", bufs=1, space="PSUM"))

        # per-k tiles, DMAs interleaved in first-use order; WT rides the
        # gpsimd (SWDGE) queue in parallel with sync's HWDGE loads
        hqT_r = hqT_d.ap().rearrange("(k p) q -> k p q", p=128)
        hpT_r = hpT_d.ap().rearrange("(k p) q -> k p q", p=128)
        WT_r = WT_d.ap().rearrange("(k p) v -> k p v", p=128)
        hpT_s = [consts.tile([128, PB], f32, name=f"hpT{k}") for k in range(KH)]
        hqT_s = [consts.tile([128, LQ], f16, name=f"hqT{k}") for k in range(KH)]
        WT_s = [consts.tile([128, V], f16, name=f"WT{k}") for k in range(KH)]
        # spread the load DMAs over four issue queues so the per-issue
        # overhead doesn't serialize the startup; each queue loads one k-set
        dma_eng = [nc.sync, nc.scalar, nc.sync, nc.scalar]
        for k in range(KH):
            dma_eng[k].dma_start(hpT_s[k][:], hpT_r[k])
            dma_eng[k].dma_start(hqT_s[k][:], hqT_r[k])
            dma_eng[k].dma_start(WT_s[k][:], WT_r[k])
        vw_s = consts.tile([128, MQ * V], f16)
        nc.scalar.dma_start(vw_s[:], vwb_d.ap())
        hq_s = consts.tile([128, MQ, H], f16)
        nc.sync.dma_start(hq_s[:], hq_d.ap().rearrange("(m p) h -> p m h", p=128))
        ident = consts.tile([128, 128], f32)
        make_identity(nc, ident[:])
        # scores[q, m, p]: column p filled per main-loop iteration
        scores = consts.tile([128, MQ, PB], f32)

        def epilogue_chunk(c0, csz):
            """softmax over q + attention output for p-rows [c0, c0+csz)."""
            s_pq = epi.tile([csz, LQ], f32, name=f"s_pq{c0}", tag="s_pq")
            for m in range(MQ):
                pst = psum_tp.tile([csz, 128], f32, tag="tp")
                nc.tensor.transpose(
                    pst[:], scores[:, m, c0 : c0 + csz], ident[:]
                )
                nc.vector.tensor_copy(s_pq[:, bass.ts(m, 128)], pst[:])
            # no max-subtraction: |s| is bounded well inside fp32 exp range
            # for this problem; softmax is shift-invariant so this matches
            # the stable-softmax reference up to rounding
            e_t = epi.tile([csz, LQ], f32, name=f"e_t{c0}", tag="e_t")
            ssum = epi.tile([csz, 1], f32, name=f"ssum{c0}", tag="ssum")
            nc.scalar.activation(e_t[:], s_pq[:], AF.Exp, accum_out=ssum[:])
            rcp = epi.tile([csz, 1], f32, name=f"rcp{c0}", tag="rcp")
            nc.vector.reciprocal(rcp[:], ssum[:])
            eT = epi.tile([128, MQ, csz], f16, name=f"eT{c0}", tag="eT")
            for m in range(MQ):
                pet = psum_tp.tile([128, csz], f32, tag="tp")
                nc.tensor.transpose(
                    pet[:], e_t[:, bass.ts(m, 128)], ident[:csz, :csz]
                )
                nc.vector.tensor_copy(eT[:, m, :], pet[:])
            out_ps = psum_out.tile([csz, H], f32, tag="outps")
            for m in range(MQ):
                nc.tensor.matmul(
                    out_ps[:],
                    eT[:, m, :],
                    hq_s[:, m, :],
                    start=(m == 0),
                    stop=(m == MQ - 1),
                )
            out_s = epi.tile([csz, H], f32, name=f"out_s{c0}", tag="out_s")
            nc.scalar.activation(out_s[:], out_ps[:], AF.Copy, scale=rcp[:])
            nc.sync.dma_start(out_d.ap()[c0 : c0 + csz, :], out_s[:])

        for p in range(PB * repeat):
            p = p % PB
            scaled = [
                scaled_pool.tile([128, LQ], f16, name=f"sc{k}_{p}", tag=f"scl{k}")
                for k in range(KH)
            ]
            for k in range(KH):
                # k=2 prep runs on ACT, except the first few p where ACT is
                # still issuing its share of the input DMAs
                if k == 2 and p >= 3:
                    nc.scalar.mul(
                        scaled[k][:], hqT_s[k][:], hpT_s[k][:, p : p + 1]
                    )
                else:
                    nc.gpsimd.tensor_scalar_mul(
                        scaled[k][:], hqT_s[k][:], hpT_s[k][:, p : p + 1]
                    )
            ps = psum_main.tile([128, MQ * V], f32, tag="ps")
            for m in range(MQ):
                for k in range(KH):
                    nc.tensor.matmul(
                        ps[:, m * V : (m + 1) * V],
                        scaled[k][:, bass.ts(m, 128)],
                        WT_s[k][:],
                        start=(k == 0),
                        stop=(k == KH - 1),
                    )
            tw = tanh_pool.tile([128, MQ * V], f16, tag="tw")
            sc = scratch_pool.tile([128, MQ, V], f16, tag="sc")
            if p < PB - 2:
                nc.scalar.activation(tw[:], ps[:], AF.Tanh)
                nc.vector.tensor_mul(
                    sc[:].rearrange("p m v -> p (m v)"), tw[:], vw_s[:]
                )
                nc.vector.reduce_sum(
                    scores[:, :, p : p + 1], sc[:], axis=mybir.AxisListType.X
                )
            else:
                # tail latency: split by m so DVE starts on m=0 while ACT
                # still computes m=1's tanh
                for m in range(MQ):
                    nc.scalar.activation(
                        tw[:, m * V : (m + 1) * V],
                        ps[:, m * V : (m + 1) * V],
                        AF.Tanh,
                    )
                    nc.vector.tensor_mul(
                        sc[:, m, :], tw[:, m * V : (m + 1) * V],
                        vw_s[:, m * V : (m + 1) * V],
                    )
                    nc.vector.reduce_sum(
                        scores[:, m, p : p + 1], sc[:, m, :],
                        axis=mybir.AxisListType.X,
                    )
            if (p + 1) % EPI_CHUNK == 0:
                epilogue_chunk(p + 1 - EPI_CHUNK, EPI_CHUNK)

    nc.compile()
    return nc


def get_nc():
    global _CACHED_NC
    if _CACHED_NC is None:
        _CACHED_NC = _build_nc()
    return _CACHED_NC


def make_in_maps(hq, hp, W, v_w):
    hq = np.asarray(hq, dtype=np.float32)
    hp = np.asarray(hp, dtype=np.float32)
    W = np.asarray(W, dtype=np.float32)
    v_w = np.asarray(v_w, dtype=np.float32)
    WT = np.ascontiguousarray(W.T).astype(np.float16)
    vw1 = v_w.reshape(1, V).astype(np.float16)
    vwb = np.ascontiguousarray(
        np.broadcast_to(np.tile(vw1, (1, MQ)), (128, MQ * V))
    )
    in_maps = []
    for c in range(NCORES):
        b = c // 2
        pb = (c % 2) * PB
        in_maps.append(
            {
                "hqT": np.ascontiguousarray(hq[b].T).astype(np.float16),
                "hpT": np.ascontiguousarray(hp[b, pb : pb + PB].T),
                "WT": WT,
                "vwb": vwb,
                "hq": np.ascontiguousarray(hq[b]).astype(np.float16),
            }
        )
    return in_maps


def gather_out(results):
    out = np.empty((B, LP, H), np.float32)
    for c in range(NCORES):
        b = c // 2
        pb = (c % 2) * PB
        out[b, pb : pb + PB] = results[c]["out"]
    return out


def kernel(hq, hp, W, v_w):
    from concourse.bass_utils import run_bass_kernel_spmd

    nc = get_nc()
    in_maps = make_in_maps(hq, hp, W, v_w)
    res = run_bass_kernel_spmd(nc, in_maps, core_ids=list(range(NCORES)))
    return gather_out(res.results)


# revision 37
# speedup vs baseline: 1.2500x; 1.0114x over previous
"""Trainium2 Bass kernel for nn_DotAttentionUnit.

Reference computation (per batch b):
    h_mul[p,q,h] = hq[q,h] * hp[p,h]
    s_w = tanh(h_mul @ W.T)            # [p,q,v]
    s[p,q] = s_w . v_w                 # reduce over v
    a = softmax(s, axis=q)
    out[p,h] = sum_q a[p,q] * hq[q,h]

Shapes: B=4, LQ=256, LP=256, H=512, V=512.

Sharding: pure data parallel over (b, p-block): 8 cores = 4 batches x 2
p-blocks of 128. Each core computes out[b, pblk:pblk+128, :]. No
collectives.

Per-core device algorithm (PE-bound, fp16 matmul operands with fp32 PSUM
accumulation; fp16 mantissa ~ TF32, keeps rel err ~1e-4):
  for p in 0..127:
    scaled[k]  = hqT[k] * hpT[k][:, p]     (2 on Pool, 1 on ACT, 1 on Pool)
    psum[m]    = sum_k scaled[k][:,m*128:].T @ WT[k]  (PE, 8 matmuls N=512)
    tw         = tanh(psum)                (ACT, one [128,1024] op)
    sc         = tw * vw                   (DVE, one wide fp16 mul)
    scores[:, :, p] = reduce(sc)           (DVE, one fused wide reduce)
  epilogue (x2 chunks of 64 p-rows, first chunk overlapped mid-loop):
  PE-transpose scores chunk -> exp+sum (ACT, no max shift needed: |s| is
  small) -> transpose exp -> exp^T @ hq -> scale rows by 1/sum -> DMA out.
"""

import numpy as np

B, LQ, LP, H, V = 4, 256, 256, 512, 512
NCORES = 8
PB = 128  # p rows per core
KH = H // 128  # 4 contraction tiles
MQ = LQ // 128  # 2 q tiles
EPI_CHUNK = 64

_CACHED_NC = None


def _build_nc(repeat=1):
    from contextlib import ExitStack

    import concourse.bass as bass
    import concourse.mybir as mybir
    import concourse.tile as tile
    from concourse import bacc
    from concourse.masks import make_identity

    f32 = mybir.dt.float32
    f16 = mybir.dt.float16
    AF = mybir.ActivationFunctionType

    nc = bacc.Bacc("TRN2", target_bir_lowering=False, debug=False)

    hqT_d = nc.dram_tensor("hqT", [H, LQ], f16, kind="ExternalInput")
    hpT_d = nc.dram_tensor("hpT", [H, PB], f32, kind="ExternalInput")
    WT_d = nc.dram_tensor("WT", [H, V], f16, kind="ExternalInput")
    vwb_d = nc.dram_tensor("vwb", [128, MQ * V], f16, kind="ExternalInput")
    hq_d = nc.dram_tensor("hq", [LQ, H], f16, kind="ExternalInput")
    out_d = nc.dram_tensor("out", [PB, H], f32, kind="ExternalOutput")

    with tile.TileContext(nc) as tc, ExitStack() as ctx:
        consts = ctx.enter_context(tc.tile_pool(name="consts", bufs=1))
        scaled_pool = ctx.enter_context(tc.tile_pool(name="scaled", bufs=4))
        tanh_pool = ctx.enter_context(tc.tile_pool(name="tanh", bufs=4))
        scratch_pool = ctx.enter_context(tc.tile_pool(name="scratch", bufs=3))
        epi = ctx.enter_context(tc.tile_pool(name="epi", bufs=2))
        psum_main = ctx.enter_context(
            tc.tile_pool(name="psmain", bufs=2, space="PSUM")
        )
        psum_tp = ctx.enter_context(tc.tile_pool(name="pstp", bufs=2, space="PSUM"))
        psum_out = ctx.enter_context(tc.tile_pool(name="psout", bufs=1, space="PSUM"))

        # per-k tiles, DMAs interleaved in first-use order; WT rides the
        # gpsimd (SWDGE) queue in parallel with sync's HWDGE loads
        hqT_r = hqT_d.ap().rearrange("(k p) q -> k p q", p=128)
        hpT_r = hpT_d.ap().rearrange("(k p) q -> k p q", p=128)
        WT_r = WT_d.ap().rearrange("(k p) v -> k p v", p=128)
        hpT_s = [consts.tile([128, PB], f32, name=f"hpT{k}") for k in range(KH)]
        hqT_s = [consts.tile([128, LQ], f16, name=f"hqT{k}") for k in range(KH)]
        WT_s = [consts.tile([128, V], f16, name=f"WT{k}") for k in range(KH)]
        # spread the load DMAs over four issue queues so the per-issue
        # overhead doesn't serialize the startup; each queue loads one k-set
        dma_eng = [nc.sync, nc.scalar, nc.sync, nc.scalar]
        for k in range(KH):
            dma_eng[k].dma_start(hpT_s[k][:], hpT_r[k])
            dma_eng[k].dma_start(hqT_s[k][:], hqT_r[k])
            dma_eng[k].dma_start(WT_s[k][:], WT_r[k])
        vw_s = consts.tile([128, MQ * V], f16)
        nc.scalar.dma_start(vw_s[:], vwb_d.ap())
        hq_s = consts.tile([128, MQ, H], f16)
        nc.sync.dma_start(hq_s[:], hq_d.ap().rearrange("(m p) h -> p m h", p=128))
        ident = consts.tile([128, 128], f32)
        make_identity(nc, ident[:])
        # scores[q, m, p]: column p filled per main-loop iteration
        scores = consts.tile([128, MQ, PB], f32)

        # PE warmup: dummy matmuls on a zeroed tile fill the otherwise-idle
        # input-DMA window so the PE clock (HAM) and p-state are at full
        # speed when the first real matmul issues
        wz = consts.tile([128, V], f16, name="wz")
        nc.vector.memset(wz[:], 0.0)
        wps = psum_tp.tile([128, V], f32, tag="tp")
        N_WARM = 6
        for i in range(N_WARM):
            nc.tensor.matmul(
                wps[:], wz[:, :128], wz[:], start=(i == 0), stop=(i == N_WARM - 1)
            )
        wtr = consts.tile([128, V], f32, name="wtr")
        nc.vector.tensor_copy(wtr[:], wps[:])

        def epilogue_chunk(c0, csz):
            """softmax over q + attention output for p-rows [c0, c0+csz)."""
            s_pq = epi.tile([csz, LQ], f32, name=f"s_pq{c0}", tag="s_pq")
            for m in range(MQ):
                pst = psum_tp.tile([csz, 128], f32, tag="tp")
                nc.tensor.transpose(
                    pst[:], scores[:, m, c0 : c0 + csz], ident[:]
                )
                nc.vector.tensor_copy(s_pq[:, bass.ts(m, 128)], pst[:])
            # no max-subtraction: |s| is bounded well inside fp32 exp range
            # for this problem; softmax is shift-invariant so this matches
            # the stable-softmax reference up to rounding
            e_t = epi.tile([csz, LQ], f32, name=f"e_t{c0}", tag="e_t")
            ssum = epi.tile([csz, 1], f32, name=f"ssum{c0}", tag="ssum")
            nc.scalar.activation(e_t[:], s_pq[:], AF.Exp, accum_out=ssum[:])
            rcp = epi.tile([csz, 1], f32, name=f"rcp{c0}", tag="rcp")
            nc.vector.reciprocal(rcp[:], ssum[:])
            eT = epi.tile([128, MQ, csz], f16, name=f"eT{c0}", tag="eT")
            for m in range(MQ):
                pet = psum_tp.tile([128, csz], f32, tag="tp")
                nc.tensor.transpose(
                    pet[:], e_t[:, bass.ts(m, 128)], ident[:csz, :csz]
                )
                nc.vector.tensor_copy(eT[:, m, :], pet[:])
            out_ps = psum_out.tile([csz, H], f32, tag="outps")
            for m in range(MQ):
                nc.tensor.matmul(
                    out_ps[:],
                    eT[:, m, :],
                    hq_s[:, m, :],
                    start=(m == 0),
                    stop=(m == MQ - 1),
                )
            out_s = epi.tile([csz, H], f32, name=f"out_s{c0}", tag="out_s")
            nc.scalar.activation(out_s[:], out_ps[:], AF.Copy, scale=rcp[:])
            nc.sync.dma_start(out_d.ap()[c0 : c0 + csz, :], out_s[:])

        for p in range(PB * repeat):
            p = p % PB
            scaled = [
                scaled_pool.tile([128, LQ], f16, name=f"sc{k}_{p}", tag=f"scl{k}")
                for k in range(KH)
            ]
            for k in range(KH):
                # k=2 prep runs on ACT, except the first few p where ACT is
                # still issuing its share of the input DMAs
                if k == 2 and p >= 3:
                    nc.scalar.mul(
                        scaled[k][:], hqT_s[k][:], hpT_s[k][:, p : p + 1]
                    )
                else:
                    nc.gpsimd.tensor_scalar_mul(
                        scaled[k][:], hqT_s[k][:], hpT_s[k][:, p : p + 1]
                    )
            ps = psum_main.tile([128, MQ * V], f32, tag="ps")
            for m in range(MQ):
                for k in range(KH):
                    nc.tensor.matmul(
                        ps[:, m * V : (m + 1) * V],
                        scaled[k][:, bass.ts(m, 128)],
                        WT_s[k][:],
                        start=(k == 0),
                        stop=(k == KH - 1),
                    )
            tw = tanh_pool.tile([128, MQ * V], f16, tag="tw")
            sc = scratch_pool.tile([128, MQ, V], f16, tag="sc")
            if p < PB - 2:
                nc.scalar.activation(tw[:], ps[:], AF.Tanh)
                nc.vector.tensor_mul(
                    sc[:].rearrange("p m v -> p (m v)"), tw[:], vw_s[:]
                )
                if p % 4 == 3:
                    # DVE sits within ~1% of PE's per-p budget; every 4th p
                    # hand the m=1 half-reduce to ACT (Identity + accum_out)
                    # so neither engine rides the knife edge
                    nc.vector.reduce_sum(
                        scores[:, 0, p : p + 1], sc[:, 0, :],
                        axis=mybir.AxisListType.X,
                    )
                    trash = scratch_pool.tile([128, V], f16, tag="trash")
                    nc.scalar.activation(
                        trash[:], sc[:, 1, :], AF.Identity,
                        accum_out=scores[:, 1, p : p + 1],
                    )
                else:
                    nc.vector.reduce_sum(
                        scores[:, :, p : p + 1], sc[:],
                        axis=mybir.AxisListType.X,
                    )
            else:
                # tail latency: split by m so DVE starts on m=0 while ACT
                # still computes m=1's tanh
                for m in range(MQ):
                    nc.scalar.activation(
                        tw[:, m * V : (m + 1) * V],
                        ps[:, m * V : (m + 1) * V],
                        AF.Tanh,
                    )
                    nc.vector.tensor_mul(
                        sc[:, m, :], tw[:, m * V : (m + 1) * V],
                        vw_s[:, m * V : (m + 1) * V],
                    )
                    nc.vector.reduce_sum(
                        scores[:, m, p : p + 1], sc[:, m, :],
                        axis=mybir.AxisListType.X,
                    )
            if (p + 1) % EPI_CHUNK == 0:
                epilogue_chunk(p + 1 - EPI_CHUNK, EPI_CHUNK)

    nc.compile()
    return nc


def get_nc():
    global _CACHED_NC
    if _CACHED_NC is None:
        _CACHED_NC = _build_nc()
    return _CACHED_NC


def make_in_maps(hq, hp, W, v_w):
    hq = np.asarray(hq, dtype=np.float32)
    hp = np.asarray(hp, dtype=np.float32)
    W = np.asarray(W, dtype=np.float32)
    v_w = np.asarray(v_w, dtype=np.float32)
    WT = np.ascontiguousarray(W.T).astype(np.float16)
    vw1 = v_w.reshape(1, V).astype(np.float16)
    vwb = np.ascontiguousarray(
        np.broadcast_to(np.tile(vw1, (1, MQ)), (128, MQ * V))
    )
    in_maps = []
    for c in range(NCORES):
        b = c // 2
        pb = (c % 2) * PB
        in_maps.append(
            {
                "hqT": np.ascontiguousarray(hq[b].T).astype(np.float16),
                "hpT": np.ascontiguousarray(hp[b, pb : pb + PB].T),
                "WT": WT,
                "vwb": vwb,
                "hq": np.ascontiguousarray(hq[b]).astype(np.float16),
            }
        )
    return in_maps


def gather_out(results):
    out = np.empty((B, LP, H), np.float32)
    for c in range(NCORES):
        b = c // 2
        pb = (c % 2) * PB
        out[b, pb : pb + PB] = results[c]["out"]
    return out


def kernel(hq, hp, W, v_w):
    from concourse.bass_utils import run_bass_kernel_spmd

    nc = get_nc()
    in_maps = make_in_maps(hq, hp, W, v_w)
    res = run_bass_kernel_spmd(nc, in_maps, core_ids=list(range(NCORES)))
    return gather_out(res.results)


# revision 44
# speedup vs baseline: 1.2570x; 1.0056x over previous
"""Trainium2 Bass kernel for nn_DotAttentionUnit.

Reference computation (per batch b):
    h_mul[p,q,h] = hq[q,h] * hp[p,h]
    s_w = tanh(h_mul @ W.T)            # [p,q,v]
    s[p,q] = s_w . v_w                 # reduce over v
    a = softmax(s, axis=q)
    out[p,h] = sum_q a[p,q] * hq[q,h]

Shapes: B=4, LQ=256, LP=256, H=512, V=512.

Sharding: pure data parallel over (b, p-block): 8 cores = 4 batches x 2
p-blocks of 128. Each core computes out[b, pblk:pblk+128, :]. No
collectives.

Per-core device algorithm (PE-bound, fp16 matmul operands with fp32 PSUM
accumulation; fp16 mantissa ~ TF32, keeps rel err ~1e-4):
  for p in 0..127:
    scaled[k]  = hqT[k] * hpT[k][:, p]     (2 on Pool, 1 on ACT, 1 on Pool)
    psum[m]    = sum_k scaled[k][:,m*128:].T @ WT[k]  (PE, 8 matmuls N=512)
    tw         = tanh(psum)                (ACT, one [128,1024] op)
    sc         = tw * vw                   (DVE, one wide fp16 mul)
    scores[:, :, p] = reduce(sc)           (DVE, one fused wide reduce)
  epilogue (x2 chunks of 64 p-rows, first chunk overlapped mid-loop):
  PE-transpose scores chunk -> exp+sum (ACT, no max shift needed: |s| is
  small) -> transpose exp -> exp^T @ hq -> scale rows by 1/sum -> DMA out.
"""

import numpy as np

B, LQ, LP, H, V = 4, 256, 256, 512, 512
NCORES = 8
PB = 128  # p rows per core
KH = H // 128  # 4 contraction tiles
MQ = LQ // 128  # 2 q tiles
EPI_CHUNK = 64

_CACHED_NC = None


def _build_nc(repeat=1):
    from contextlib import ExitStack

    import concourse.bass as bass
    import concourse.mybir as mybir
    import concourse.tile as tile
    from concourse import bacc
    from concourse.masks import make_identity

    f32 = mybir.dt.float32
    f16 = mybir.dt.float16
    AF = mybir.ActivationFunctionType

    nc = bacc.Bacc("TRN2", target_bir_lowering=False, debug=False)

    hqT_d = nc.dram_tensor("hqT", [H, LQ], f16, kind="ExternalInput")
    hpT_d = nc.dram_tensor("hpT", [H, PB], f32, kind="ExternalInput")
    WT_d = nc.dram_tensor("WT", [H, V], f16, kind="ExternalInput")
    vwb_d = nc.dram_tensor("vwb", [128, MQ * V], f16, kind="ExternalInput")
    hq_d = nc.dram_tensor("hq", [LQ, H], f16, kind="ExternalInput")
    out_d = nc.dram_tensor("out", [PB, H], f32, kind="ExternalOutput")

    with tile.TileContext(nc) as tc, ExitStack() as ctx:
        consts = ctx.enter_context(tc.tile_pool(name="consts", bufs=1))
        scaled_pool = ctx.enter_context(tc.tile_pool(name="scaled", bufs=4))
        tanh_pool = ctx.enter_context(tc.tile_pool(name="tanh", bufs=4))
        scratch_pool = ctx.enter_context(tc.tile_pool(name="scratch", bufs=3))
        epi = ctx.enter_context(tc.tile_pool(name="epi", bufs=2))
        psum_main = ctx.enter_context(
            tc.tile_pool(name="psmain", bufs=2, space="PSUM")
        )
        psum_tp = ctx.enter_context(tc.tile_pool(name="pstp", bufs=2, space="PSUM"))
        psum_out = ctx.enter_context(tc.tile_pool(name="psout", bufs=2, space="PSUM"))

        # per-k tiles, DMAs interleaved in first-use order; WT rides the
        # gpsimd (SWDGE) queue in parallel with sync's HWDGE loads
        hqT_r = hqT_d.ap().rearrange("(k p) q -> k p q", p=128)
        hpT_r = hpT_d.ap().rearrange("(k p) q -> k p q", p=128)
        WT_r = WT_d.ap().rearrange("(k p) v -> k p v", p=128)
        hpT_s = [consts.tile([128, PB], f32, name=f"hpT{k}") for k in range(KH)]
        hqT_s = [consts.tile([128, LQ], f16, name=f"hqT{k}") for k in range(KH)]
        WT_s = [consts.tile([128, V], f16, name=f"WT{k}") for k in range(KH)]
        # spread the load DMAs over four issue queues so the per-issue
        # overhead doesn't serialize the startup; each queue loads one k-set
        dma_eng = [nc.sync, nc.scalar, nc.sync, nc.scalar]
        for k in range(KH):
            dma_eng[k].dma_start(hpT_s[k][:], hpT_r[k])
            dma_eng[k].dma_start(hqT_s[k][:], hqT_r[k])
            if k < 2:
                dma_eng[k].dma_start(WT_s[k][:], WT_r[k])
            else:
                # third queue: Pool is idle early (first preps run on DVE),
                # so its SWDGE issue slot parallelizes the two late W tiles
                nc.gpsimd.dma_start(WT_s[k][:], WT_r[k])
        vw_s = consts.tile([128, MQ * V], f16)
        nc.scalar.dma_start(vw_s[:], vwb_d.ap())
        hq_s = consts.tile([128, MQ, H], f16)
        nc.sync.dma_start(hq_s[:], hq_d.ap().rearrange("(m p) h -> p m h", p=128))
        ident = consts.tile([128, 128], f32)
        make_identity(nc, ident[:])
        # scores[q, m, p]: column p filled per main-loop iteration
        scores = consts.tile([128, MQ, PB], f32)

        # PE warmup: dummy matmuls on a zeroed tile fill the otherwise-idle
        # input-DMA window so the PE clock (HAM) and p-state are at full
        # speed when the first real matmul issues
        wz = consts.tile([128, V], f16, name="wz")
        nc.vector.memset(wz[:], 0.0)
        wps = psum_tp.tile([128, V], f32, tag="tp")
        N_WARM = 6
        for i in range(N_WARM):
            nc.tensor.matmul(
                wps[:], wz[:, :128], wz[:], start=(i == 0), stop=(i == N_WARM - 1)
            )
        wtr = consts.tile([128, V], f32, name="wtr")
        nc.vector.tensor_copy(wtr[:], wps[:])

        def epilogue_chunk(c0, csz):
            """softmax over q + attention output for p-rows [c0, c0+csz)."""
            s_pq = epi.tile([csz, LQ], f32, name=f"s_pq{c0}", tag="s_pq")
            for m in range(MQ):
                pst = psum_tp.tile([csz, 128], f32, tag="tp")
                nc.tensor.transpose(
                    pst[:], scores[:, m, c0 : c0 + csz], ident[:]
                )
                nc.vector.tensor_copy(s_pq[:, bass.ts(m, 128)], pst[:])
            # no max-subtraction: |s| is bounded well inside fp32 exp range
            # for this problem; softmax is shift-invariant so this matches
            # the stable-softmax reference up to rounding
            e_t = epi.tile([csz, LQ], f32, name=f"e_t{c0}", tag="e_t")
            ssum = epi.tile([csz, 1], f32, name=f"ssum{c0}", tag="ssum")
            nc.scalar.activation(e_t[:], s_pq[:], AF.Exp, accum_out=ssum[:])
            rcp = epi.tile([csz, 1], f32, name=f"rcp{c0}", tag="rcp")
            nc.vector.reciprocal(rcp[:], ssum[:])
            eT = epi.tile([128, MQ, csz], f16, name=f"eT{c0}", tag="eT")
            for m in range(MQ):
                pet = psum_tp.tile([128, csz], f32, tag="tp")
                nc.tensor.transpose(
                    pet[:], e_t[:, bass.ts(m, 128)], ident[:csz, :csz]
                )
                nc.vector.tensor_copy(eT[:, m, :], pet[:])
            out_ps = psum_out.tile([csz, H], f32, tag="outps")
            for m in range(MQ):
                nc.tensor.matmul(
                    out_ps[:],
                    eT[:, m, :],
                    hq_s[:, m, :],
                    start=(m == 0),
                    stop=(m == MQ - 1),
                )
            out_s = epi.tile([csz, H], f32, name=f"out_s{c0}", tag="out_s")
            nc.scalar.activation(out_s[:], out_ps[:], AF.Copy, scale=rcp[:])
            nc.sync.dma_start(out_d.ap()[c0 : c0 + csz, :], out_s[:])

        for p in range(PB * repeat):
            p = p % PB
            scaled = [
                scaled_pool.tile([128, LQ], f16, name=f"sc{k}_{p}", tag=f"scl{k}")
                for k in range(KH)
            ]
            for k in range(KH):
                # steady state: k=2 on ACT, rest on Pool. For the first few
                # p, ACT is still issuing input DMAs and Pool's serial preps
                # would starve the PE — run those preps on the idle DVE
                # (fp16 single-src tensor_scalar is 4x-mode there, ~127ns)
                if p < 6:
                    nc.vector.tensor_scalar_mul(
                        scaled[k][:], hqT_s[k][:], hpT_s[k][:, p : p + 1]
                    )
                elif k == 2:
                    nc.scalar.mul(
                        scaled[k][:], hqT_s[k][:], hpT_s[k][:, p : p + 1]
                    )
                else:
                    nc.gpsimd.tensor_scalar_mul(
                        scaled[k][:], hqT_s[k][:], hpT_s[k][:, p : p + 1]
                    )
            ps = psum_main.tile([128, MQ * V], f32, tag="ps")
            for m in range(MQ):
                for k in range(KH):
                    nc.tensor.matmul(
                        ps[:, m * V : (m + 1) * V],
                        scaled[k][:, bass.ts(m, 128)],
                        WT_s[k][:],
                        start=(k == 0),
                        stop=(k == KH - 1),
                    )
            tw = tanh_pool.tile([128, MQ * V], f16, tag="tw")
            sc = scratch_pool.tile([128, MQ, V], f16, tag="sc")
            if p < PB - 2:
                nc.scalar.activation(tw[:], ps[:], AF.Tanh)
                nc.vector.tensor_mul(
                    sc[:].rearrange("p m v -> p (m v)"), tw[:], vw_s[:]
                )
                if p % 4 == 3:
                    # DVE sits within ~1% of PE's per-p budget; every 4th p
                    # hand the m=1 half-reduce to ACT (Identity + accum_out)
                    # so neither engine rides the knife edge
                    nc.vector.reduce_sum(
                        scores[:, 0, p : p + 1], sc[:, 0, :],
                        axis=mybir.AxisListType.X,
                    )
                    trash = scratch_pool.tile([128, V], f16, tag="trash")
                    nc.scalar.activation(
                        trash[:], sc[:, 1, :], AF.Identity,
                        accum_out=scores[:, 1, p : p + 1],
                    )
                else:
                    nc.vector.reduce_sum(
                        scores[:, :, p : p + 1], sc[:],
                        axis=mybir.AxisListType.X,
                    )
            else:
                # tail latency: split by m so DVE starts on m=0 while ACT
                # still computes m=1's tanh
                for m in range(MQ):
                    nc.scalar.activation(
                        tw[:, m * V : (m + 1) * V],
                        ps[:, m * V : (m + 1) * V],
                        AF.Tanh,
                    )
                    nc.vector.tensor_mul(
                        sc[:, m, :], tw[:, m * V : (m + 1) * V],
                        vw_s[:, m * V : (m + 1) * V],
                    )
                    nc.vector.reduce_sum(
                        scores[:, m, p : p + 1], sc[:, m, :],
                        axis=mybir.AxisListType.X,
                    )
            if (p + 1) % EPI_CHUNK == 0:
                epilogue_chunk(p + 1 - EPI_CHUNK, EPI_CHUNK)

    nc.compile()
    return nc


def get_nc():
    global _CACHED_NC
    if _CACHED_NC is None:
        _CACHED_NC = _build_nc()
    return _CACHED_NC


def make_in_maps(hq, hp, W, v_w):
    hq = np.asarray(hq, dtype=np.float32)
    hp = np.asarray(hp, dtype=np.float32)
    W = np.asarray(W, dtype=np.float32)
    v_w = np.asarray(v_w, dtype=np.float32)
    WT = np.ascontiguousarray(W.T).astype(np.float16)
    vw1 = v_w.reshape(1, V).astype(np.float16)
    vwb = np.ascontiguousarray(
        np.broadcast_to(np.tile(vw1, (1, MQ)), (128, MQ * V))
    )
    in_maps = []
    for c in range(NCORES):
        b = c // 2
        pb = (c % 2) * PB
        in_maps.append(
            {
                "hqT": np.ascontiguousarray(hq[b].T).astype(np.float16),
                "hpT": np.ascontiguousarray(hp[b, pb : pb + PB].T),
                "WT": WT,
                "vwb": vwb,
                "hq": np.ascontiguousarray(hq[b]).astype(np.float16),
            }
        )
    return in_maps


def gather_out(results):
    out = np.empty((B, LP, H), np.float32)
    for c in range(NCORES):
        b = c // 2
        pb = (c % 2) * PB
        out[b, pb : pb + PB] = results[c]["out"]
    return out


def kernel(hq, hp, W, v_w):
    from concourse.bass_utils import run_bass_kernel_spmd

    nc = get_nc()
    in_maps = make_in_maps(hq, hp, W, v_w)
    res = run_bass_kernel_spmd(nc, in_maps, core_ids=list(range(NCORES)))
    return gather_out(res.results)


# revision 45
# speedup vs baseline: 1.2621x; 1.0040x over previous
"""Trainium2 Bass kernel for nn_DotAttentionUnit.

Reference computation (per batch b):
    h_mul[p,q,h] = hq[q,h] * hp[p,h]
    s_w = tanh(h_mul @ W.T)            # [p,q,v]
    s[p,q] = s_w . v_w                 # reduce over v
    a = softmax(s, axis=q)
    out[p,h] = sum_q a[p,q] * hq[q,h]

Shapes: B=4, LQ=256, LP=256, H=512, V=512.

Sharding: pure data parallel over (b, p-block): 8 cores = 4 batches x 2
p-blocks of 128. Each core computes out[b, pblk:pblk+128, :]. No
collectives.

Per-core device algorithm (PE-bound, fp16 matmul operands with fp32 PSUM
accumulation; fp16 mantissa ~ TF32, keeps rel err ~1e-4):
  for p in 0..127:
    scaled[k]  = hqT[k] * hpT[k][:, p]     (2 on Pool, 1 on ACT, 1 on Pool)
    psum[m]    = sum_k scaled[k][:,m*128:].T @ WT[k]  (PE, 8 matmuls N=512)
    tw         = tanh(psum)                (ACT, one [128,1024] op)
    sc         = tw * vw                   (DVE, one wide fp16 mul)
    scores[:, :, p] = reduce(sc)           (DVE, one fused wide reduce)
  epilogue (x2 chunks of 64 p-rows, first chunk overlapped mid-loop):
  PE-transpose scores chunk -> exp+sum (ACT, no max shift needed: |s| is
  small) -> transpose exp -> exp^T @ hq -> scale rows by 1/sum -> DMA out.
"""

import numpy as np

B, LQ, LP, H, V = 4, 256, 256, 512, 512
NCORES = 8
PB = 128  # p rows per core
KH = H // 128  # 4 contraction tiles
MQ = LQ // 128  # 2 q tiles
EPI_CHUNK = 64

_CACHED_NC = None


def _build_nc(repeat=1):
    from contextlib import ExitStack

    import concourse.bass as bass
    import concourse.mybir as mybir
    import concourse.tile as tile
    from concourse import bacc
    from concourse.masks import make_identity

    f32 = mybir.dt.float32
    f16 = mybir.dt.float16
    AF = mybir.ActivationFunctionType

    nc = bacc.Bacc("TRN2", target_bir_lowering=False, debug=False)

    hqT_d = nc.dram_tensor("hqT", [H, LQ], f16, kind="ExternalInput")
    hpT_d = nc.dram_tensor("hpT", [H, PB], f32, kind="ExternalInput")
    WT_d = nc.dram_tensor("WT", [H, V], f16, kind="ExternalInput")
    vwb_d = nc.dram_tensor("vwb", [128, MQ * V], f16, kind="ExternalInput")
    hq_d = nc.dram_tensor("hq", [LQ, H], f16, kind="ExternalInput")
    out_d = nc.dram_tensor("out", [PB, H], f32, kind="ExternalOutput")

    with tile.TileContext(nc) as tc, ExitStack() as ctx:
        consts = ctx.enter_context(tc.tile_pool(name="consts", bufs=1))
        scaled_pool = ctx.enter_context(tc.tile_pool(name="scaled", bufs=4))
        tanh_pool = ctx.enter_context(tc.tile_pool(name="tanh", bufs=4))
        scratch_pool = ctx.enter_context(tc.tile_pool(name="scratch", bufs=3))
        epi = ctx.enter_context(tc.tile_pool(name="epi", bufs=2))
        psum_main = ctx.enter_context(
            tc.tile_pool(name="psmain", bufs=2, space="PSUM")
        )
        psum_tp = ctx.enter_context(tc.tile_pool(name="pstp", bufs=2, space="PSUM"))
        psum_out = ctx.enter_context(tc.tile_pool(name="psout", bufs=2, space="PSUM"))

        # per-k tiles, DMAs interleaved in first-use order; WT rides the
        # gpsimd (SWDGE) queue in parallel with sync's HWDGE loads
        hqT_r = hqT_d.ap().rearrange("(k p) q -> k p q", p=128)
        hpT_r = hpT_d.ap().rearrange("(k p) q -> k p q", p=128)
        WT_r = WT_d.ap().rearrange("(k p) v -> k p v", p=128)
        hpT_s = [consts.tile([128, PB], f32, name=f"hpT{k}") for k in range(KH)]
        hqT_s = [consts.tile([128, LQ], f16, name=f"hqT{k}") for k in range(KH)]
        WT_s = [consts.tile([128, V], f16, name=f"WT{k}") for k in range(KH)]
        # spread the load DMAs over four issue queues so the per-issue
        # overhead doesn't serialize the startup; each queue loads one k-set
        dma_eng = [nc.sync, nc.scalar, nc.sync, nc.scalar]
        for k in range(KH):
            dma_eng[k].dma_start(hpT_s[k][:], hpT_r[k])
            dma_eng[k].dma_start(hqT_s[k][:], hqT_r[k])
            if k < 2:
                dma_eng[k].dma_start(WT_s[k][:], WT_r[k])
            else:
                # third queue: Pool is idle early (first preps run on DVE),
                # so its SWDGE issue slot parallelizes the two late W tiles
                nc.gpsimd.dma_start(WT_s[k][:], WT_r[k])
        vw_s = consts.tile([128, MQ * V], f16)
        nc.scalar.dma_start(vw_s[:], vwb_d.ap())
        hq_s = consts.tile([128, MQ, H], f16)
        nc.sync.dma_start(hq_s[:], hq_d.ap().rearrange("(m p) h -> p m h", p=128))
        ident = consts.tile([128, 128], f32)
        make_identity(nc, ident[:])
        # scores[q, m, p]: column p filled per main-loop iteration
        scores = consts.tile([128, MQ, PB], f32)

        # PE warmup: dummy matmuls on a zeroed tile fill the otherwise-idle
        # input-DMA window so the PE clock (HAM) and p-state are at full
        # speed when the first real matmul issues
        wz = consts.tile([128, V], f16, name="wz")
        nc.vector.memset(wz[:], 0.0)
        wps = psum_tp.tile([128, V], f32, tag="tp")
        N_WARM = 8
        for i in range(N_WARM):
            nc.tensor.matmul(
                wps[:], wz[:, :128], wz[:], start=(i == 0), stop=(i == N_WARM - 1)
            )
        wtr = consts.tile([128, V], f32, name="wtr")
        nc.vector.tensor_copy(wtr[:], wps[:])

        def epilogue_chunk(c0, csz):
            """softmax over q + attention output for p-rows [c0, c0+csz)."""
            s_pq = epi.tile([csz, LQ], f32, name=f"s_pq{c0}", tag="s_pq")
            for m in range(MQ):
                pst = psum_tp.tile([csz, 128], f32, tag="tp")
                nc.tensor.transpose(
                    pst[:], scores[:, m, c0 : c0 + csz], ident[:]
                )
                nc.vector.tensor_copy(s_pq[:, bass.ts(m, 128)], pst[:])
            # no max-subtraction: |s| is bounded well inside fp32 exp range
            # for this problem; softmax is shift-invariant so this matches
            # the stable-softmax reference up to rounding
            e_t = epi.tile([csz, LQ], f32, name=f"e_t{c0}", tag="e_t")
            ssum = epi.tile([csz, 1], f32, name=f"ssum{c0}", tag="ssum")
            nc.scalar.activation(e_t[:], s_pq[:], AF.Exp, accum_out=ssum[:])
            rcp = epi.tile([csz, 1], f32, name=f"rcp{c0}", tag="rcp")
            nc.vector.reciprocal(rcp[:], ssum[:])
            eT = epi.tile([128, MQ, csz], f16, name=f"eT{c0}", tag="eT")
            for m in range(MQ):
                pet = psum_tp.tile([128, csz], f32, tag="tp")
                nc.tensor.transpose(
                    pet[:], e_t[:, bass.ts(m, 128)], ident[:csz, :csz]
                )
                nc.vector.tensor_copy(eT[:, m, :], pet[:])
            out_ps = psum_out.tile([csz, H], f32, tag="outps")
            for m in range(MQ):
                nc.tensor.matmul(
                    out_ps[:],
                    eT[:, m, :],
                    hq_s[:, m, :],
                    start=(m == 0),
                    stop=(m == MQ - 1),
                )
            out_s = epi.tile([csz, H], f32, name=f"out_s{c0}", tag="out_s")
            nc.scalar.activation(out_s[:], out_ps[:], AF.Copy, scale=rcp[:])
            nc.sync.dma_start(out_d.ap()[c0 : c0 + csz, :], out_s[:])

        for p in range(PB * repeat):
            p = p % PB
            scaled = [
                scaled_pool.tile([128, LQ], f16, name=f"sc{k}_{p}", tag=f"scl{k}")
                for k in range(KH)
            ]
            for k in range(KH):
                # steady state: k=2 on ACT, rest on Pool. For the first few
                # p, ACT is still issuing input DMAs and Pool's serial preps
                # would starve the PE — run those preps on the idle DVE
                # (fp16 single-src tensor_scalar is 4x-mode there, ~127ns)
                if p < 6:
                    nc.vector.tensor_scalar_mul(
                        scaled[k][:], hqT_s[k][:], hpT_s[k][:, p : p + 1]
                    )
                elif k == 2:
                    nc.scalar.mul(
                        scaled[k][:], hqT_s[k][:], hpT_s[k][:, p : p + 1]
                    )
                else:
                    nc.gpsimd.tensor_scalar_mul(
                        scaled[k][:], hqT_s[k][:], hpT_s[k][:, p : p + 1]
                    )
            ps = psum_main.tile([128, MQ * V], f32, tag="ps")
            for m in range(MQ):
                for k in range(KH):
                    nc.tensor.matmul(
                        ps[:, m * V : (m + 1) * V],
                        scaled[k][:, bass.ts(m, 128)],
                        WT_s[k][:],
                        start=(k == 0),
                        stop=(k == KH - 1),
                    )
            tw = tanh_pool.tile([128, MQ * V], f16, tag="tw")
            sc = scratch_pool.tile([128, MQ, V], f16, tag="sc")
            if p < PB - 2:
                nc.scalar.activation(tw[:], ps[:], AF.Tanh)
                nc.vector.tensor_mul(
                    sc[:].rearrange("p m v -> p (m v)"), tw[:], vw_s[:]
                )
                if p % 4 == 3:
                    # DVE sits within ~1% of PE's per-p budget; every 4th p
                    # hand the m=1 half-reduce to ACT (Identity + accum_out)
                    # so neither engine rides the knife edge
                    nc.vector.reduce_sum(
                        scores[:, 0, p : p + 1], sc[:, 0, :],
                        axis=mybir.AxisListType.X,
                    )
                    trash = scratch_pool.tile([128, V], f16, tag="trash")
                    nc.scalar.activation(
                        trash[:], sc[:, 1, :], AF.Identity,
                        accum_out=scores[:, 1, p : p + 1],
                    )
                else:
                    nc.vector.reduce_sum(
                        scores[:, :, p : p + 1], sc[:],
                        axis=mybir.AxisListType.X,
                    )
            else:
                # tail latency: split by m so DVE starts on m=0 while ACT
                # still computes m=1's tanh; m=1's reduce rides ACT so the
                # two half-chains finish in parallel
                for m in range(MQ):
                    nc.scalar.activation(
                        tw[:, m * V : (m + 1) * V],
                        ps[:, m * V : (m + 1) * V],
                        AF.Tanh,
                    )
                    nc.vector.tensor_mul(
                        sc[:, m, :], tw[:, m * V : (m + 1) * V],
                        vw_s[:, m * V : (m + 1) * V],
                    )
                    if m == 0:
                        nc.vector.reduce_sum(
                            scores[:, m, p : p + 1], sc[:, m, :],
                            axis=mybir.AxisListType.X,
                        )
                    else:
                        trash = scratch_pool.tile([128, V], f16, tag="trash")
                        nc.scalar.activation(
                            trash[:], sc[:, m, :], AF.Identity,
                            accum_out=scores[:, m, p : p + 1],
                        )
            if (p + 1) % EPI_CHUNK == 0:
                epilogue_chunk(p + 1 - EPI_CHUNK, EPI_CHUNK)

    nc.compile()
    return nc


def get_nc():
    global _CACHED_NC
    if _CACHED_NC is None:
        _CACHED_NC = _build_nc()
    return _CACHED_NC


def make_in_maps(hq, hp, W, v_w):
    hq = np.asarray(hq, dtype=np.float32)
    hp = np.asarray(hp, dtype=np.float32)
    W = np.asarray(W, dtype=np.float32)
    v_w = np.asarray(v_w, dtype=np.float32)
    WT = np.ascontiguousarray(W.T).astype(np.float16)
    vw1 = v_w.reshape(1, V).astype(np.float16)
    vwb = np.ascontiguousarray(
        np.broadcast_to(np.tile(vw1, (1, MQ)), (128, MQ * V))
    )
    in_maps = []
    for c in range(NCORES):
        b = c // 2
        pb = (c % 2) * PB
        in_maps.append(
            {
                "hqT": np.ascontiguousarray(hq[b].T).astype(np.float16),
                "hpT": np.ascontiguousarray(hp[b, pb : pb + PB].T),
                "WT": WT,
                "vwb": vwb,
                "hq": np.ascontiguousarray(hq[b]).astype(np.float16),
            }
        )
    return in_maps


def gather_out(results):
    out = np.empty((B, LP, H), np.float32)
    for c in range(NCORES):
        b = c // 2
        pb = (c % 2) * PB
        out[b, pb : pb + PB] = results[c]["out"]
    return out


def kernel(hq, hp, W, v_w):
    from concourse.bass_utils import run_bass_kernel_spmd

    nc = get_nc()
    in_maps = make_in_maps(hq, hp, W, v_w)
    res = run_bass_kernel_spmd(nc, in_maps, core_ids=list(range(NCORES)))
    return gather_out(res.results)
